# revision 29
# baseline (speedup 1.0000x reference)
"""Trainium2 Bass kernel for nn_AdditiveDTMGP (dense_mlp, 8 cores, data parallel).

Math (per layer): out[n,w] = sum_{d,m} exp(-|x[n,d]-u[m]|) @ Rinv . Ws[d,:,w] + bias
Folding Rinv and the Bayesian reparam into the weights on host gives
    out[n,w] = sum_{d,m} phi[n,d,m] * RW[d,m,w] + bias[w],  RW[d] = Rinv @ Ws[d]

Device pipeline per 512-sample chunk, per layer (dm-domain padded to 8*64=512,
split into 4 partition-tiles of 128 = 2 features x 64 gridpoints):
    PE   : t[p,n] = x[d(p),n] - u[m(p)]   (selector lhsT rows 0-7 + "-u" row 8
                                           against rhs with a ones-row, K=9)
    DVE  : |t| = bits(t) & 0x7fffffff     (single all-bitwise tensor_scalar;
                                           HW allows only 1 sem-wait & no
                                           arith+bitwise dual op, and abs_max
                                           is not a legal HW ALU op)
    ACT  : phi = Exp(-|t|)
    PE   : h[w,n] += RW_tile.T @ phi      (4-tile PSUM accumulation)
    ACT  : copy h PSUM->SBUF (rhs of the next layer must live in SBUF)
Layer bias enters via pad row 63 where phi==1 by construction (selector and
-u both zero there); the same row generates the next layer's ones-row as an
extra output column of RW. KL terms are batch-independent -> host numpy.
"""

import numpy as np

N, D, M = 131072, 8, 63
NCORES = 8
NC_N = N // NCORES          # 16384 samples per core
W1, W2, W3 = 8, 8, 10
MP = 64                     # padded gridpoints per feature
DM = D * MP                 # 512 padded (d,m) rows
NT = DM // 128              # 4 partition tiles
F = 512                     # chunk free size (samples per chunk)
NCHUNK = NC_N // F          # 32
KB = D + 1                  # broadcast contraction size (features + ones row)
# per-layer (weight columns, output rows): layers 1-2 carry a 9th ones column
LW = {1: W1 + 1, 2: W2 + 1, 3: W3}
CSELU = DM // 2             # selu stored as f16 pairs: 512 f16 = 256 f32 cols
CBLOB = CSELU + NT * (LW[1] + LW[2] + LW[3])  # 256 + 36+36+40 = 368

_CACHE = {}


def _softplus(x):
    return np.log1p(np.exp(x))


def _host_prep(inputs):
    """Fold Rinv + Bayesian reparam into padded weight tables; compute KL."""
    f32 = np.float32
    U = (np.arange(1, M + 1, dtype=f32) / f32(M + 1)).astype(f32)
    K = np.exp(-np.abs(U[:, None] - U[None, :])).astype(f32)
    L = np.linalg.cholesky(K).astype(f32)
    try:
        from scipy.linalg import solve_triangular
        Rinv = solve_triangular(L, np.eye(M, dtype=f32), lower=True).T.astype(f32)
    except ImportError:
        Rinv = np.linalg.inv(L.astype(np.float64)).T.astype(f32)

    kl_total = np.float64(0.0)
    rws = {}
    for l, w in ((1, W1), (2, W2), (3, W3)):
        Wmu = np.asarray(inputs[f"W{l}_mu"], f32)
        Wrho = np.asarray(inputs[f"W{l}_rho"], f32)
        bmu = np.asarray(inputs[f"b{l}_mu"], f32)
        brho = np.asarray(inputs[f"b{l}_rho"], f32)
        eW = np.asarray(inputs[f"eps_W{l}"], f32)
        eb = np.asarray(inputs[f"eps_b{l}"], f32)
        sW = _softplus(Wrho)
        sb = _softplus(brho)
        Ws = Wmu + sW * eW                       # [D, M, w]
        bs = bmu + sb * eb                       # [D, w]
        RW = np.einsum("mk,dkw->dmw", Rinv, Ws)  # [D, M, w]
        wl = LW[l]
        rw_pad = np.zeros((DM, wl), f32)
        for d in range(D):
            rw_pad[d * MP:d * MP + M, :w] = RW[d]
        rw_pad[M, :w] = bs.sum(axis=0)           # bias via phi==1 pad row
        if l < 3:
            rw_pad[M, w] = 1.0                   # ones-row generator column
        rws[l] = rw_pad.reshape(NT, 128, wl).transpose(1, 0, 2).reshape(128, NT * wl)
        kl64 = lambda mu, s: np.sum(
            0.5 * (s.astype(np.float64) ** 2 + mu.astype(np.float64) ** 2)
            - np.log(s.astype(np.float64)) - 0.5)
        kl_total += kl64(Wmu, sW) + kl64(bmu, sb)

    # selector+u matrix [KB, DM]: rows 0-7 pick the feature, row 8 carries -u;
    # all-zero at pad rows (p%64==63) so t=0 and phi=1 there
    selu = np.zeros((KB, DM), f32)
    for p in range(DM):
        i = p % MP
        if i != M:
            selu[p // MP, p] = 1.0
            selu[D, p] = -(i + 1) / f32(MP)
    blob = np.zeros((128, CBLOB), f32)
    # selu stored float16 (sel 0/1 and u=m/64 are exact); 2 f16 per f32 cell
    blob[:KB, 0:CSELU] = np.ascontiguousarray(
        selu.astype(np.float16)).view(np.float32)
    col = CSELU
    for l in (1, 2, 3):
        blob[:, col:col + NT * LW[l]] = rws[l]
        col += NT * LW[l]
    return {"blob": blob}, np.float32(kl_total)


def _build_nc():
    import concourse.bass as bass
    import concourse.mybir as mybir
    import concourse.tile as tile

    f32 = mybir.dt.float32
    f16 = mybir.dt.float16
    i32 = mybir.dt.int32
    AF = mybir.ActivationFunctionType
    OP = mybir.AluOpType

    nc = bass.Bass()
    xt_d = nc.declare_dram_parameter("xt", [KB, NC_N], f16, isOutput=False)
    blob_d = nc.declare_dram_parameter("blob", [128, CBLOB], f32, isOutput=False)
    out_d = nc.declare_dram_parameter("out", [W3, NC_N], f32, isOutput=True)

    with tile.TileContext(nc) as tc:
        with (
            tc.tile_pool(name="singles", bufs=1) as singles,
            tc.tile_pool(name="bc", bufs=6, space="PSUM") as bc_pool,
            tc.tile_pool(name="hps", bufs=2, space="PSUM") as hps_pool,
            tc.tile_pool(name="ts", bufs=3) as ts_pool,
            tc.tile_pool(name="phi", bufs=3) as phi_pool,
            tc.tile_pool(name="hsb", bufs=NCHUNK + 2, space="SBUF") as hsb_pool,
        ):
            blob_s = singles.tile([128, CBLOB], f32)
            nc.sync.dma_start(out=blob_s, in_=blob_d[:, :])
            selu_s = blob_s[0:KB, 0:CSELU].bitcast(f16)
            rw_s = {}
            col = CSELU
            for l in (1, 2, 3):
                rw_s[l] = blob_s[:, col:col + NT * LW[l]].rearrange(
                    "p (t w) -> p t w", t=NT)
                col += NT * LW[l]
            xt_s = singles.tile([KB, NC_N], f16)
            for q in range(4):
                qs = q * (NC_N // 4)
                qe = (q + 1) * (NC_N // 4)
                nc.sync.dma_start(out=xt_s[:, qs:qe], in_=xt_d[:, qs:qe])

            # Dummy matmul touching only blob: makes PE observe the blob DMA
            # semaphore here, so the first real matmul waits only on the xt
            # queue (one DMA-event wait per PE instruction).
            scratch = hps_pool.tile([D, D], f32, tag="hps")
            nc.tensor.matmul(scratch, lhsT=blob_s[0:D, 0:D],
                             rhs=blob_s[0:D, 0:D], start=True, stop=True)

            # layer-major: all chunks of a layer are independent, so the
            # three engines pipeline across chunks with no serial chain
            hs_prev = {}
            for l in (1, 2, 3):
                wl = LW[l]
                hs_cur = {}
                for c in range(NCHUNK):
                    src = (xt_s[:, c * F:(c + 1) * F] if l == 1 else hs_prev[c])
                    # broadcast matmuls: four single-bank psum tiles
                    tsi = ts_pool.tile([128, 4 * F], i32, tag="ts")
                    for t in range(NT):
                        bc = bc_pool.tile([128, F], f32, tag="bc", name=f"bc{t}")
                        nc.tensor.matmul(
                            bc, lhsT=selu_s[:, t * 128:(t + 1) * 128],
                            rhs=src, start=True, stop=True)
                        # |t| as a single all-bitwise op on the int32 view
                        nc.vector.tensor_scalar(
                            out=tsi[:, t * F:(t + 1) * F], in0=bc.bitcast(i32),
                            scalar1=0x7FFFFFFF, scalar2=None, op0=OP.bitwise_and)
                    # phi = exp(-|t|) per dm-tile, contraction interleaved
                    phi_t = phi_pool.tile([128, 4 * F], f32, tag="phi")
                    hp = hps_pool.tile([wl, F], f32, tag="hps")
                    for t in range(NT):
                        sl = slice(t * F, (t + 1) * F)
                        nc.scalar.activation(out=phi_t[:, sl],
                                             in_=tsi[:, sl].bitcast(f32),
                                             func=AF.Exp, scale=-1.0)
                        nc.tensor.matmul(
                            hp, lhsT=rw_s[l][:, t, :],
                            rhs=phi_t[:, sl],
                            start=(t == 0), stop=(t == NT - 1))
                    # move h psum -> sbuf (ACT copy; DMA cannot read PSUM)
                    hs = hsb_pool.tile([wl, F], f16 if l < 3 else f32, tag="hsb")
                    nc.scalar.activation(out=hs, in_=hp, func=AF.Copy)
                    if l < 3:
                        hs_cur[c] = hs
                    else:
                        nc.sync.dma_start(out=out_d[:, c * F:(c + 1) * F], in_=hs)
                hs_prev = hs_cur

    # split multi-sem waits to satisfy the 1-wait-per-instruction HW limit
    # (these run in Bacc.compile(); plain Bass.finalize() skips them)
    import bass_rust
    bass_rust.move_matmul_waits_to_ldweights(nc.m)
    bass_rust.generate_event_semaphores(nc)
    return nc


def _get_nc():
    if "nc" not in _CACHE:
        _CACHE["nc"] = _build_nc()
    return _CACHE["nc"]


def _run_device(in_maps, trace=False):
    from concourse.bass_utils import run_bass_kernel_spmd
    nc = _get_nc()
    return run_bass_kernel_spmd(nc, in_maps, core_ids=list(range(NCORES)),
                                trace=trace)


def kernel(**inputs):
    consts, kl = _host_prep(inputs)
    x = np.asarray(inputs["x"], np.float32)          # [N, D]
    xT = np.ascontiguousarray(x.T)                   # [D, N]
    in_maps = []
    for i in range(NCORES):
        xa = np.empty((KB, NC_N), np.float16)
        xa[:D] = xT[:, i * NC_N:(i + 1) * NC_N].astype(np.float16)
        xa[D] = 1.0
        m = {"xt": xa}
        m.update(consts)
        in_maps.append(m)
    res = _run_device(in_maps, trace=False)
    outs = [res.results[i]["out"] for i in range(NCORES)]  # each [10, NC_N]
    h = np.concatenate(outs, axis=1).T                     # [N, 10]
    return np.ascontiguousarray(h, np.float32), np.float32(kl)


# revision 34
# speedup vs baseline: 1.0005x; 1.0005x over previous
"""Trainium2 Bass kernel for nn_AdditiveDTMGP (dense_mlp, 8 cores, data parallel).

Math (per layer): out[n,w] = sum_{d,m} exp(-|x[n,d]-u[m]|) @ Rinv . Ws[d,:,w] + bias
Folding Rinv and the Bayesian reparam into the weights on host gives
    out[n,w] = sum_{d,m} phi[n,d,m] * RW[d,m,w] + bias[w],  RW[d] = Rinv @ Ws[d]

Device pipeline per 512-sample chunk, per layer (dm-domain padded to 8*64=512,
split into 4 partition-tiles of 128 = 2 features x 64 gridpoints):
    PE   : t[p,n] = x[d(p),n] - u[m(p)]   (selector lhsT rows 0-7 + "-u" row 8
                                           against rhs with a ones-row, K=9)
    DVE  : |t| = bits(t) & 0x7fffffff     (single all-bitwise tensor_scalar;
                                           HW allows only 1 sem-wait & no
                                           arith+bitwise dual op, and abs_max
                                           is not a legal HW ALU op)
    ACT  : phi = Exp(-|t|)
    PE   : h[w,n] += RW_tile.T @ phi      (4-tile PSUM accumulation)
    ACT  : copy h PSUM->SBUF (rhs of the next layer must live in SBUF)
Layer bias enters via pad row 63 where phi==1 by construction (selector and
-u both zero there); the same row generates the next layer's ones-row as an
extra output column of RW. KL terms are batch-independent -> host numpy.
"""

import numpy as np

N, D, M = 131072, 8, 63
NCORES = 8
NC_N = N // NCORES          # 16384 samples per core
W1, W2, W3 = 8, 8, 10
MP = 64                     # padded gridpoints per feature
DM = D * MP                 # 512 padded (d,m) rows
NT = DM // 128              # 4 partition tiles
F = 512                     # chunk free size (samples per chunk)
NCHUNK = NC_N // F          # 32
KB = D + 1                  # broadcast contraction size (features + ones row)
# per-layer (weight columns, output rows): layers 1-2 carry a 9th ones column
LW = {1: W1 + 1, 2: W2 + 1, 3: W3}
CSELU = DM // 2             # selu stored as f16 pairs: 512 f16 = 256 f32 cols
CBLOB = CSELU + NT * (LW[1] + LW[2] + LW[3])  # 256 + 36+36+40 = 368

_CACHE = {}


def _softplus(x):
    return np.log1p(np.exp(x))


def _host_prep(inputs):
    """Fold Rinv + Bayesian reparam into padded weight tables; compute KL."""
    f32 = np.float32
    U = (np.arange(1, M + 1, dtype=f32) / f32(M + 1)).astype(f32)
    K = np.exp(-np.abs(U[:, None] - U[None, :])).astype(f32)
    L = np.linalg.cholesky(K).astype(f32)
    try:
        from scipy.linalg import solve_triangular
        Rinv = solve_triangular(L, np.eye(M, dtype=f32), lower=True).T.astype(f32)
    except ImportError:
        Rinv = np.linalg.inv(L.astype(np.float64)).T.astype(f32)

    kl_total = np.float64(0.0)
    rws = {}
    for l, w in ((1, W1), (2, W2), (3, W3)):
        Wmu = np.asarray(inputs[f"W{l}_mu"], f32)
        Wrho = np.asarray(inputs[f"W{l}_rho"], f32)
        bmu = np.asarray(inputs[f"b{l}_mu"], f32)
        brho = np.asarray(inputs[f"b{l}_rho"], f32)
        eW = np.asarray(inputs[f"eps_W{l}"], f32)
        eb = np.asarray(inputs[f"eps_b{l}"], f32)
        sW = _softplus(Wrho)
        sb = _softplus(brho)
        Ws = Wmu + sW * eW                       # [D, M, w]
        bs = bmu + sb * eb                       # [D, w]
        RW = np.einsum("mk,dkw->dmw", Rinv, Ws)  # [D, M, w]
        wl = LW[l]
        rw_pad = np.zeros((DM, wl), f32)
        for d in range(D):
            rw_pad[d * MP:d * MP + M, :w] = RW[d]
        rw_pad[M, :w] = bs.sum(axis=0)           # bias via phi==1 pad row
        if l < 3:
            rw_pad[M, w] = 1.0                   # ones-row generator column
        rws[l] = rw_pad.reshape(NT, 128, wl).transpose(1, 0, 2).reshape(128, NT * wl)
        kl64 = lambda mu, s: np.sum(
            0.5 * (s.astype(np.float64) ** 2 + mu.astype(np.float64) ** 2)
            - np.log(s.astype(np.float64)) - 0.5)
        kl_total += kl64(Wmu, sW) + kl64(bmu, sb)

    # selector+u matrix [KB, DM]: rows 0-7 pick the feature, row 8 carries -u;
    # all-zero at pad rows (p%64==63) so t=0 and phi=1 there
    selu = np.zeros((KB, DM), f32)
    for p in range(DM):
        i = p % MP
        if i != M:
            selu[p // MP, p] = 1.0
            selu[D, p] = -(i + 1) / f32(MP)
    blob = np.zeros((128, CBLOB), f32)
    # selu stored float16 (sel 0/1 and u=m/64 are exact); 2 f16 per f32 cell
    blob[:KB, 0:CSELU] = np.ascontiguousarray(
        selu.astype(np.float16)).view(np.float32)
    col = CSELU
    for l in (1, 2, 3):
        blob[:, col:col + NT * LW[l]] = rws[l]
        col += NT * LW[l]
    return {"blob": blob}, np.float32(kl_total)


def _build_nc():
    import concourse.bass as bass
    import concourse.mybir as mybir
    import concourse.tile as tile

    f32 = mybir.dt.float32
    f16 = mybir.dt.float16
    i32 = mybir.dt.int32
    AF = mybir.ActivationFunctionType
    OP = mybir.AluOpType

    nc = bass.Bass()
    xt_d = nc.declare_dram_parameter("xt", [KB, NC_N], f16, isOutput=False)
    blob_d = nc.declare_dram_parameter("blob", [128, CBLOB], f32, isOutput=False)
    out_d = nc.declare_dram_parameter("out", [W3, NC_N], f32, isOutput=True)

    with tile.TileContext(nc) as tc:
        with (
            tc.tile_pool(name="singles", bufs=1) as singles,
            tc.tile_pool(name="bc", bufs=6, space="PSUM") as bc_pool,
            tc.tile_pool(name="hps", bufs=2, space="PSUM") as hps_pool,
            tc.tile_pool(name="ts", bufs=4) as ts_pool,
            tc.tile_pool(name="phi", bufs=4) as phi_pool,
            tc.tile_pool(name="hsb", bufs=NCHUNK + 2, space="SBUF") as hsb_pool,
        ):
            blob_s = singles.tile([128, CBLOB], f32)
            nc.sync.dma_start(out=blob_s, in_=blob_d[:, :])
            selu_s = blob_s[0:KB, 0:CSELU].bitcast(f16)
            rw_s = {}
            col = CSELU
            for l in (1, 2, 3):
                rw_s[l] = blob_s[:, col:col + NT * LW[l]].rearrange(
                    "p (t w) -> p t w", t=NT)
                col += NT * LW[l]
            xt_s = singles.tile([KB, NC_N], f16)
            for q in range(4):
                qs = q * (NC_N // 4)
                qe = (q + 1) * (NC_N // 4)
                nc.sync.dma_start(out=xt_s[:, qs:qe], in_=xt_d[:, qs:qe])

            # Dummy matmul touching only blob: makes PE observe the blob DMA
            # semaphore here, so the first real matmul waits only on the xt
            # queue (one DMA-event wait per PE instruction).
            scratch = hps_pool.tile([D, D], f32, tag="hps")
            nc.tensor.matmul(scratch, lhsT=blob_s[0:D, 0:D],
                             rhs=blob_s[0:D, 0:D], start=True, stop=True)

            # layer-major: all chunks of a layer are independent, so the
            # three engines pipeline across chunks with no serial chain
            hs_prev = {}
            for l in (1, 2, 3):
                wl = LW[l]
                hs_cur = {}
                for c in range(NCHUNK):
                    src = (xt_s[:, c * F:(c + 1) * F] if l == 1 else hs_prev[c])
                    # broadcast matmuls: four single-bank psum tiles
                    tsi = ts_pool.tile([128, 4 * F], i32, tag="ts")
                    for t in range(NT):
                        bc = bc_pool.tile([128, F], f32, tag="bc", name=f"bc{t}")
                        nc.tensor.matmul(
                            bc, lhsT=selu_s[:, t * 128:(t + 1) * 128],
                            rhs=src, start=True, stop=True)
                        # |t| as a single all-bitwise op on the int32 view
                        nc.vector.tensor_scalar(
                            out=tsi[:, t * F:(t + 1) * F], in0=bc.bitcast(i32),
                            scalar1=0x7FFFFFFF, scalar2=None, op0=OP.bitwise_and)
                    # phi = exp(-|t|) per dm-tile, contraction interleaved
                    phi_t = phi_pool.tile([128, 4 * F], f32, tag="phi")
                    hp = hps_pool.tile([wl, F], f32, tag="hps")
                    for t in range(NT):
                        sl = slice(t * F, (t + 1) * F)
                        nc.scalar.activation(out=phi_t[:, sl],
                                             in_=tsi[:, sl].bitcast(f32),
                                             func=AF.Exp, scale=-1.0)
                        nc.tensor.matmul(
                            hp, lhsT=rw_s[l][:, t, :],
                            rhs=phi_t[:, sl],
                            start=(t == 0), stop=(t == NT - 1))
                    # move h psum -> sbuf (ACT copy; DMA cannot read PSUM)
                    hs = hsb_pool.tile([wl, F], f16 if l < 3 else f32, tag="hsb")
                    nc.scalar.activation(out=hs, in_=hp, func=AF.Copy)
                    if l < 3:
                        hs_cur[c] = hs
                    else:
                        nc.sync.dma_start(out=out_d[:, c * F:(c + 1) * F], in_=hs)
                hs_prev = hs_cur

    # split multi-sem waits to satisfy the 1-wait-per-instruction HW limit
    # (these run in Bacc.compile(); plain Bass.finalize() skips them)
    import bass_rust
    bass_rust.move_matmul_waits_to_ldweights(nc.m)
    bass_rust.generate_event_semaphores(nc)
    return nc


def _get_nc():
    if "nc" not in _CACHE:
        _CACHE["nc"] = _build_nc()
    return _CACHE["nc"]


def _run_device(in_maps, trace=False):
    from concourse.bass_utils import run_bass_kernel_spmd
    nc = _get_nc()
    return run_bass_kernel_spmd(nc, in_maps, core_ids=list(range(NCORES)),
                                trace=trace)


def kernel(**inputs):
    consts, kl = _host_prep(inputs)
    x = np.asarray(inputs["x"], np.float32)          # [N, D]
    xT = np.ascontiguousarray(x.T)                   # [D, N]
    in_maps = []
    for i in range(NCORES):
        xa = np.empty((KB, NC_N), np.float16)
        xa[:D] = xT[:, i * NC_N:(i + 1) * NC_N].astype(np.float16)
        xa[D] = 1.0
        m = {"xt": xa}
        m.update(consts)
        in_maps.append(m)
    res = _run_device(in_maps, trace=False)
    outs = [res.results[i]["out"] for i in range(NCORES)]  # each [10, NC_N]
    h = np.concatenate(outs, axis=1).T                     # [N, 10]
    return np.ascontiguousarray(h, np.float32), np.float32(kl)
